# revision 7
# baseline (speedup 1.0000x reference)
"""MoE layer (B=4,S=2048,D=1024,F=2048,E=8,topK=2, softmax over token axis)
for 8 Trainium2 NeuronCores.

Strategy: balanced expert parallelism with sparse token dispatch, bf16.
 - Host: gating matmul (jax-CPU for bit-exact selection), top-2, softmax over
   the token axis, per-expert token gather.
 - Load balance: core e owns expert e's first <=2048 tokens (segment A,
   4x512-token blocks); overflow tokens spill in <=128-token chunks to the
   cores' fixed 128-token segment B (own core first, then cores whose expert
   did not overflow, which load a second expert's weights).
 - All matmul operands bf16 (f32 PSUM accumulation); hT kept bf16 in SBUF.
 - Host packs x/w1/w2 into SBUF-layout tensors with wide rows (8-32KB per
   partition line) because DMA throughput is row-descriptor-limited
   (~4.7ns/row): a [128, n*512] bf16 tile with 1KB rows moves at half the
   rate of the same bytes with 4KB rows.
 - Host: scatter-add the 8 outputs back to [B,S,D].
"""
import os
import sys

for _p in ("/opt/trn_rl_repo", "/root/.axon_site/_ro/trn_rl_repo"):
    if os.path.isdir(_p) and _p not in sys.path:
        sys.path.append(_p)

import numpy as np
import ml_dtypes
import concourse.bass as bass
import concourse.mybir as mybir
from concourse.tile import TileContext
from concourse.bass_utils import run_bass_kernel_spmd

B, S, D, F, E, K = 4, 2048, 1024, 2048, 8, 2
N = B * S
P = 128
ND = D // P           # 8 d-tiles
NF = F // P           # 16 f-tiles
FQ = F // 4           # 512-wide f quarter
SEG_A = 2048          # per-core primary segment (4 x 512-token blocks)
SEG_B = 128           # per-core spill segment (1 x 128-token block)
R = SEG_A + SEG_B     # tokens processed per core
DT = mybir.dt.bfloat16
NPDT = ml_dtypes.bfloat16

_cache = {}


def _split_sync_waits(nc, max_waits=1):
    """The walrus build in this env rejects instructions carrying more than
    ~1 sync wait (Matmult S3_LW: 1; Drain: <3). Hoist extra waits onto
    same-engine NOPs placed immediately before the offending instruction —
    semantically identical (engine executes waits in order)."""
    ctr = 0
    for f in nc.m.functions:
        for blk in f.blocks:
            new_list = []
            changed = False
            for inst in blk.instructions:
                si = inst.sync_info
                ow = list(si.on_wait) if si and si.on_wait else []
                if len(ow) > max_waits:
                    extra, keep = ow[:-max_waits], ow[-max_waits:]
                    for i in range(0, len(extra), max_waits):
                        ctr += 1
                        nop = mybir.InstNoOp(
                            name=f"I-waitsplit-{ctr}",
                            engine=inst.engine,
                            sync_info=mybir.SyncInfo(
                                on_wait=list(extra[i:i + max_waits]), on_update=[]
                            ),
                        )
                        new_list.append(nop)
                    si.on_wait = keep
                    inst.sync_info = si
                    changed = True
                new_list.append(inst)
            if changed:
                blk.instructions = new_list


# xt SBUF/host layout: per block b (4x512 + 1x128 tokens), per d-tile,
# token-minor: col(b, d, t) = off_b + d*tb_b + t
_BLOCKS = [(0, 512), (512, 512), (1024, 512), (1536, 512), (2048, 128)]
_XOFF = []
_o = 0
for _base, _tb in _BLOCKS:
    _XOFF.append(_o)
    _o += ND * _tb
XT_COLS = _o                      # 17408
W1_COLS = ND * F                  # (q, d, fr128) layout: q*4*... = 4 quarters
W2_COLS = NF * D                  # (dh, f, c) layout


def _build_balanced():
    """Per-core program: 4x512-token blocks with expert-A weights + 1x128
    spill block with expert-B weights; 2176 tokens total."""
    nc = bass.Bass("TRN2", target_bir_lowering=False, debug=False, num_devices=E)

    xt_d = nc.dram_tensor("xt", [P, XT_COLS], DT, kind="ExternalInput")
    w1a_d = nc.dram_tensor("w1a", [P, W1_COLS], DT, kind="ExternalInput")
    w2a_d = nc.dram_tensor("w2a", [P, W2_COLS], DT, kind="ExternalInput")
    w1b_d = nc.dram_tensor("w1b", [P, W1_COLS], DT, kind="ExternalInput")
    w2b_d = nc.dram_tensor("w2b", [P, W2_COLS], DT, kind="ExternalInput")
    b1a_d = nc.dram_tensor("b1a", [P, NF], mybir.dt.float32, kind="ExternalInput")
    b1b_d = nc.dram_tensor("b1b", [P, NF], mybir.dt.float32, kind="ExternalInput")
    wgtc_d = nc.dram_tensor("wgtc", [P, R // P], mybir.dt.float32, kind="ExternalInput")
    y_d = nc.dram_tensor("y", [R, D], mybir.dt.float32, kind="ExternalOutput")

    Relu = mybir.ActivationFunctionType.Relu
    Copy = mybir.ActivationFunctionType.Copy

    with TileContext(nc) as tc:
        with tc.tile_pool(name="sb", bufs=1) as sbpool, \
             tc.tile_pool(name="ypool", bufs=4) as ypool, \
             tc.tile_pool(name="ps1", bufs=4, space="PSUM") as ps1pool, \
             tc.tile_pool(name="ps2", bufs=4, space="PSUM") as ps2pool:

            xt = sbpool.tile([P, XT_COLS], DT, tag="xt")
            w1a = sbpool.tile([P, W1_COLS], DT, tag="w1a")
            w2a = sbpool.tile([P, W2_COLS], DT, tag="w2a")
            w1b = sbpool.tile([P, W1_COLS], DT, tag="w1b")
            w2b = sbpool.tile([P, W2_COLS], DT, tag="w2b")

            # DMA issue order = consumption order; wide rows so each DMA
            # streams at full rate. The f=0 chain's weights (fr-block 0 of
            # quarter 0, 256KB) + block-0 x first, then the rest of w1 at
            # fr-block granularity so chains unblock as the stream lands.
            FRB = ND * P  # cols per (quarter, fr-block): 8 d x 128
            nc.sync.dma_start(out=w1a[:, :FRB], in_=w1a_d[:, :FRB])
            nc.sync.dma_start(out=xt[:, :_XOFF[1]], in_=xt_d[:, :_XOFF[1]])

            # warm-up: keep the PE busy during the initial weight DMA so the
            # HAM clock gate is at 8/8 (2.4GHz) when real matmuls start
            warm = sbpool.tile([P, 256], DT, tag="warm")
            nc.gpsimd.memset(warm[:, :].bitcast(mybir.dt.float32), 0.0)
            ps_w = ps1pool.tile([P, 512], mybir.dt.float32, tag="ps1")
            for _ in range(20):
                nc.tensor.matmul(ps_w[:, :256], lhsT=warm[:, :P], rhs=warm[:, :],
                                 start=True, stop=True)

            b1a = sbpool.tile([P, NF], mybir.dt.float32, tag="b1a")
            nc.sync.dma_start(out=b1a[:, :], in_=b1a_d[:, :])
            b1b = sbpool.tile([P, NF], mybir.dt.float32, tag="b1b")
            nc.sync.dma_start(out=b1b[:, :], in_=b1b_d[:, :])
            wgt_sb = sbpool.tile([P, R // P], mybir.dt.float32, tag="wgt")
            nc.sync.dma_start(out=wgt_sb[:, :], in_=wgtc_d[:, :])
            for fb in range(1, 16):
                nc.sync.dma_start(out=w1a[:, fb * FRB:(fb + 1) * FRB],
                                  in_=w1a_d[:, fb * FRB:(fb + 1) * FRB])
            nc.sync.dma_start(out=w2a[:, :NF * (D // 2)], in_=w2a_d[:, :NF * (D // 2)])
            nc.sync.dma_start(out=xt[:, _XOFF[1]:_XOFF[2]], in_=xt_d[:, _XOFF[1]:_XOFF[2]])
            nc.sync.dma_start(out=w2a[:, NF * (D // 2):], in_=w2a_d[:, NF * (D // 2):])
            nc.sync.dma_start(out=xt[:, _XOFF[2]:_XOFF[3]], in_=xt_d[:, _XOFF[2]:_XOFF[3]])
            nc.sync.dma_start(out=xt[:, _XOFF[3]:_XOFF[4]], in_=xt_d[:, _XOFF[3]:_XOFF[4]])
            nc.sync.dma_start(out=xt[:, _XOFF[4]:], in_=xt_d[:, _XOFF[4]:])
            nc.sync.dma_start(out=w1b[:, :], in_=w1b_d[:, :])
            nc.sync.dma_start(out=w2b[:, :], in_=w2b_d[:, :])

            for bi, (base, tb) in enumerate(_BLOCKS):
                is_b = bi == 4
                w1s, w2s, b1s = (w1b, w2b, b1b) if is_b else (w1a, w2a, b1a)
                xoff = _XOFF[bi]
                # mm1: hT[f] = relu(sum_d w1[d,f].T @ xt[d] + b1[f])
                hT = sbpool.tile([P, NF * tb], DT, tag="hTb" if is_b else "hT")
                for f in range(NF):
                    ps = ps1pool.tile([P, 512], mybir.dt.float32, tag="ps1")
                    for d in range(ND):
                        nc.tensor.matmul(
                            ps[:, :tb],
                            lhsT=w1s[:, f * ND * P + d * P: f * ND * P + (d + 1) * P],
                            rhs=xt[:, xoff + d * tb: xoff + (d + 1) * tb],
                            start=(d == 0),
                            stop=(d == ND - 1),
                        )
                    nc.scalar.activation(
                        hT[:, f * tb:(f + 1) * tb], ps[:, :tb], Relu,
                        bias=b1s[:, f:f + 1],
                    )
                # mm2: y[tok, :] = (hT.T @ w2) * wgt[tok]
                for dh in range(2):
                    for th in range(tb // P):
                        ps2 = ps2pool.tile([P, D // 2], mybir.dt.float32, tag="ps2")
                        for f in range(NF):
                            nc.tensor.matmul(
                                ps2[:, :],
                                lhsT=hT[:, f * tb + th * P: f * tb + (th + 1) * P],
                                rhs=w2s[:, dh * NF * (D // 2) + f * (D // 2):
                                        dh * NF * (D // 2) + (f + 1) * (D // 2)],
                                start=(f == 0),
                                stop=(f == NF - 1),
                            )
                        y_sb = ypool.tile([P, D // 2], mybir.dt.float32, tag="y")
                        nc.scalar.activation(
                            y_sb[:, :], ps2[:, :], Copy,
                            scale=wgt_sb[:, base // P + th: base // P + th + 1],
                        )
                        nc.sync.dma_start(
                            out=y_d[base + th * P: base + (th + 1) * P,
                                    dh * (D // 2):(dh + 1) * (D // 2)],
                            in_=y_sb[:, :],
                        )
    _split_sync_waits(nc)
    return nc


def _x_pack(tokens_a, tokens_b, x_flat):
    """Build the [P, XT_COLS] bf16 SBUF-layout x tensor: per block, per
    d-tile, token-minor."""
    out = np.zeros((P, XT_COLS), dtype=NPDT)
    xa = np.zeros((SEG_A, D), dtype=np.float32)
    xa[:len(tokens_a)] = x_flat[tokens_a]
    # (4 blk, 512 tok, 8 d, 128 p) -> (p, blk, d, tok)
    out[:, :_XOFF[4]] = np.ascontiguousarray(
        xa.reshape(4, 512, ND, P).transpose(3, 0, 2, 1).reshape(P, ND * SEG_A)
    ).astype(NPDT)
    xb = np.zeros((SEG_B, D), dtype=np.float32)
    xb[:len(tokens_b)] = x_flat[tokens_b]
    out[:, _XOFF[4]:] = np.ascontiguousarray(
        xb.reshape(SEG_B, ND, P).transpose(2, 1, 0).reshape(P, ND * SEG_B)
    ).astype(NPDT)
    return out


def _w1_pack(w1e):
    """[D, F] -> [P, W1_COLS] with col(f, d, c) = f*ND*P + d*P + c
    (f-tile-major so mm1's chains consume the DMA stream in order)."""
    # (8 d, 128 p, 16 f, 128 c) -> (p, f, d, c)
    return np.ascontiguousarray(
        w1e.reshape(ND, P, NF, P).transpose(1, 2, 0, 3).reshape(P, W1_COLS)
    ).astype(NPDT)


def _w2_pack(w2e):
    """[F, D] -> [P, W2_COLS] with col(dh, f, c) = dh*NF*512 + f*512 + c."""
    # (16 f, 128 p, 2 dh, 512 c) -> (p, dh, f, c)
    return np.ascontiguousarray(
        w2e.reshape(NF, P, 2, D // 2).transpose(1, 2, 0, 3).reshape(P, W2_COLS)
    ).astype(NPDT)


def _routing(x_flat, gate_w):
    """Replicates: logits = x @ gate_w; top-2; softmax over token axis.
    Uses jax-CPU einsum when available so expert selection is bit-identical
    to the reference; falls back to float64 numpy."""
    try:
        import jax
        import jax.numpy as jnp
        cpu = jax.devices("cpu")[0]
        with jax.default_device(cpu):
            logits = np.asarray(
                jnp.einsum(
                    "bsd,de->bse",
                    jnp.asarray(x_flat.reshape(B, S, D)),
                    jnp.asarray(gate_w),
                )
            ).reshape(N, E)
    except Exception:
        logits = (x_flat.astype(np.float64) @ gate_w.astype(np.float64)).astype(
            np.float32
        )

    ar = np.arange(N)
    sel1 = logits.argmax(1)
    v1 = logits[ar, sel1]
    l2 = logits.copy()
    l2[ar, sel1] = -np.inf
    sel2 = l2.argmax(1)
    v2 = logits[ar, sel2]

    # softmax over the token axis per (batch, k) — matches jax.nn.softmax(axis=1)
    v = np.stack([v1, v2], 1).reshape(B, S, K)
    m = v.max(axis=1, keepdims=True)
    ev = np.exp(v - m)
    sm = (ev / ev.sum(axis=1, keepdims=True)).reshape(N, K).astype(np.float32)
    return sel1, sel2, sm[:, 0], sm[:, 1]


def _pack_bins(idx):
    """Assign each expert's overflow (tokens beyond SEG_A) to the cores'
    SEG_B bins in <=SEG_B chunks; own core's bin first. Returns per-core
    (expert, start, length) or None if it doesn't fit."""
    chunks = []
    for e in range(E):
        c = len(idx[e])
        s = SEG_A
        while s < c:
            L = min(c - s, SEG_B)
            chunks.append((e, s, L))
            s += L
    if len(chunks) > E:
        return None
    bins = [None] * E
    rest = []
    for ch in chunks:
        if bins[ch[0]] is None:
            bins[ch[0]] = ch
        else:
            rest.append(ch)
    free = [i for i in range(E) if bins[i] is None]
    for ch in rest:
        bins[free.pop(0)] = ch
    return bins


def _prepare(x, gate_w, w1, b1, w2, b2):
    x = np.ascontiguousarray(np.asarray(x, dtype=np.float32))
    gate_w = np.ascontiguousarray(np.asarray(gate_w, dtype=np.float32))
    w1 = np.asarray(w1, dtype=np.float32)
    b1 = np.asarray(b1, dtype=np.float32)
    w2 = np.asarray(w2, dtype=np.float32)
    b2 = np.asarray(b2, dtype=np.float32)

    x_flat = x.reshape(N, D)
    sel1, sel2, sm1, sm2 = _routing(x_flat, gate_w)

    idx, wgt = [], []
    for e in range(E):
        m1 = sel1 == e
        m2 = sel2 == e
        idx_e = np.nonzero(m1 | m2)[0]
        wgt_e = np.where(m1[idx_e], sm1[idx_e], sm2[idx_e]).astype(np.float32)
        idx.append(idx_e)
        wgt.append(wgt_e)

    bins = _pack_bins(idx)
    if bins is None:
        raise NotImplementedError  # caught by kernel(): extreme imbalance

    if "bal" not in _cache:
        _cache["bal"] = _build_balanced()
    nc = _cache["bal"]

    in_maps = []
    w1p = {}
    w2p = {}
    for e in range(E):
        w1p[e] = _w1_pack(w1[e])
        w2p[e] = _w2_pack(w2[e])
    for e in range(E):
        na = min(len(idx[e]), SEG_A)
        tok_a = idx[e][:na]
        be, bs, bl = bins[e] if bins[e] is not None else (e, len(idx[e]), 0)
        tok_b = idx[be][bs:bs + bl]
        wgt_full = np.zeros(R, dtype=np.float32)
        wgt_full[:na] = wgt[e][:na]
        wgt_full[SEG_A:SEG_A + bl] = wgt[be][bs:bs + bl]
        in_maps.append({
            "xt": _x_pack(tok_a, tok_b, x_flat),
            "w1a": w1p[e],
            "w2a": w2p[e],
            "w1b": w1p[be],
            "w2b": w2p[be],
            "b1a": np.ascontiguousarray(b1[e].reshape(NF, P).T),
            "b1b": np.ascontiguousarray(b1[be].reshape(NF, P).T),
            "wgtc": np.ascontiguousarray(wgt_full.reshape(R // P, P).T),
        })

    def combine(ys):
        out = np.zeros((N, D), dtype=np.float32)
        for e in range(E):
            na = min(len(idx[e]), SEG_A)
            out[idx[e][:na]] += ys[e][:na]
            if bins[e] is not None:
                be, bs, bl = bins[e]
                out[idx[be][bs:bs + bl]] += ys[e][SEG_A:SEG_A + bl]
            if b2[e].any():
                out[idx[e]] += wgt[e][:, None] * b2[e][None, :]
        return out.reshape(B, S, D)

    return nc, in_maps, combine


def kernel(x, gate_w, w1, b1, w2, b2):
    try:
        nc, in_maps, combine = _prepare(x, gate_w, w1, b1, w2, b2)
    except NotImplementedError:
        return _kernel_fallback(x, gate_w, w1, b1, w2, b2)
    res = run_bass_kernel_spmd(nc, in_maps, list(range(E)))
    return combine([res.results[e]["y"] for e in range(E)])


def _kernel_fallback(x, gate_w, w1, b1, w2, b2):
    """Dense jax/numpy fallback for pathologically imbalanced routing (an
    expert overflowing all 8 spill bins). Correct for any input."""
    x = np.asarray(x, dtype=np.float32)
    x_flat = x.reshape(N, D)
    sel1, sel2, sm1, sm2 = _routing(x_flat, np.asarray(gate_w, np.float32))
    w1 = np.asarray(w1, np.float32)
    b1 = np.asarray(b1, np.float32)
    w2 = np.asarray(w2, np.float32)
    b2 = np.asarray(b2, np.float32)
    out = np.zeros((N, D), dtype=np.float32)
    for e in range(E):
        m1 = sel1 == e
        m2 = sel2 == e
        ids = np.nonzero(m1 | m2)[0]
        if len(ids) == 0:
            continue
        w_tok = np.where(m1[ids], sm1[ids], sm2[ids]).astype(np.float32)
        h = np.maximum(x_flat[ids] @ w1[e] + b1[e], 0.0)
        out[ids] += w_tok[:, None] * (h @ w2[e] + b2[e])
    return out.reshape(B, S, D)


if __name__ == "__main__":
    rng = np.random.default_rng(0)
    inputs = {
        "x": rng.standard_normal((B, S, D)).astype(np.float32),
        "gate_w": (rng.standard_normal((D, E)) * 0.02).astype(np.float32),
        "w1": (rng.standard_normal((E, D, F)) * 0.02).astype(np.float32),
        "b1": np.zeros((E, F), np.float32),
        "w2": (rng.standard_normal((E, F, D)) * 0.02).astype(np.float32),
        "b2": np.zeros((E, D), np.float32),
    }
    out = kernel(**inputs)
    print("out", out.shape, out.dtype, np.abs(out).max())
